# revision 64
# baseline (speedup 1.0000x reference)
"""BertSelfAttention with relative-position key/value biases on 8 TRN2 NeuronCores.

Sharding: core c -> batch c//2, heads (c%2)*8 .. +8  (8 independent (b,h) pairs/core).
Per head the kernel computes scoresT[j,i] = k_j . q_i (+ banded rel-pos key bias,
inserted via GPSIMD local_scatter shear + bf16 transpose-matmuls accumulating into
PSUM), exp via ScalarE (mask bias folded into the activation bias operand), then
ctxT[d,i] = sum_j v'[j,d] probsT[j,i] where v' carries a ones-column so row 64 of
the PSUM accumulator is the softmax normalizer. The banded value term is recomputed
in [i,*] coords (narrow matmuls + exp + un-shear local_scatter) and accumulated into
the same PSUM via Wrv^T matmuls. Normalization happens on-device; the host only
shards inputs / gathers output shards (plus the constant bv offset).

v2 restructure (HAM-warmth oriented):
- single 8-bank PSUM layout alive for the whole kernel: one 4-slot transient
  pool shared by projections/a_k/scores/band/ppt/rb, plus per-side 2-bank ctx
  accumulators so both heads of an A/B pair overlap instead of serializing.
- projections stream per-k-chunk DMA and run chunked k-outer; a_k + the GPSIMD
  shear scatters are emitted right after each q chunk so they complete during
  the k/v projections.
- the banded-value chain (band scores -> exp -> un-shear -> transpose) for all
  four groups runs before PV, buffered in SBUF, so the Wrv matmuls interleaved
  into PV never stall the PE.
- side-batched band exp/scatter (one local_scatter per group instead of two).
- reciprocal_approx_fast for the softmax normalizer (5x faster than
  nc.vector.reciprocal).
"""

import os
import sys

sys.path.insert(0, "/opt/trn_rl_repo")

import numpy as np

import concourse.bass as bass
import concourse.bacc as bacc
import concourse.mybir as mybir
from concourse import bass_utils
from concourse.tile import TileContext
from concourse import library_config

F32 = mybir.dt.float32
F32R = mybir.dt.float32r
BF16 = mybir.dt.bfloat16
I16 = mybir.dt.int16
AF = mybir.ActivationFunctionType

B, S, HID, H, DH = 4, 1024, 1024, 16, 64
WK = WV = 64
NW = 2 * WK + 1  # 129
NCORES = 8
NHC = 8          # heads per core
ST = S // 128    # 8 seq tiles
KT = HID // 128  # 8 contraction tiles
NEG = -1.0e30

LAST_EXEC_NS = None
LAST_RESULTS = None


def _build_nc(with_mask_bias=False):
    nc = bacc.Bacc()

    # ---- external I/O (per-core shards) ----
    xt_e = nc.declare_dram_parameter("xt", [HID, S], F32, isOutput=False)
    wqt_e = nc.declare_dram_parameter("wqt", [HID, 512], F32, isOutput=False)
    wkt_e = nc.declare_dram_parameter("wkt", [HID, 512], F32, isOutput=False)
    wvt_e = nc.declare_dram_parameter("wvt", [HID, 512], F32, isOutput=False)
    bqc_e = nc.declare_dram_parameter("bq_cols", [128, 4], F32, isOutput=False)
    bkc_e = nc.declare_dram_parameter("bk_cols", [128, 4], F32, isOutput=False)
    wrkt_e = nc.declare_dram_parameter("wrkt", [128, 256], F32, isOutput=False)
    wrvta_e = nc.declare_dram_parameter("wrvt_a", [128, 65], F32, isOutput=False)
    wrvtb_e = nc.declare_dram_parameter("wrvt_b", [4, 65], F32, isOutput=False)
    mbp_e = nc.declare_dram_parameter("mbias_pad", [1, S + 128], F32, isOutput=False)
    mbc_e = nc.declare_dram_parameter("mbias_cols", [128, ST], F32, isOutput=False)
    iden_e = nc.declare_dram_parameter("identity", [128, 128], F32, isOutput=False)
    insx_e = nc.declare_dram_parameter("ins_idx", [128, 528], I16, isOutput=False)
    valx_e = nc.declare_dram_parameter("val_idx2", [128, 1024], I16, isOutput=False)
    ones_e = nc.declare_dram_parameter("ones_row", [128, 128], F32, isOutput=False)
    out_e = nc.declare_dram_parameter("out", [NHC, DH, S], F32, isOutput=True)

    with TileContext(nc) as tc, nc.allow_low_precision(
        reason="float32r rounding copies feeding the PE; bf16 probs/corrections"
    ):
        with (
            tc.tile_pool(name="const", bufs=1) as cpool,
            tc.tile_pool(name="persist", bufs=1) as ppool,
            tc.tile_pool(name="xw", bufs=1) as xw,
            tc.tile_pool(name="wt", bufs=1) as wtp,
            tc.tile_pool(name="probs", bufs=1) as prp,
            tc.tile_pool(name="sm", bufs=2) as smp,
            tc.tile_pool(name="stp", bufs=4, space="PSUM") as stp,
            tc.tile_pool(name="ctxp", bufs=1, space="PSUM") as ctxp,
        ):
            # ---- constants into SBUF (before x/w so the sync DMA FIFO can
            # never wedge behind weight-slot waits) ----
            wrkt_f = cpool.tile([128, 256], F32, tag="wrkt_f")
            nc.sync.dma_start(out=wrkt_f[:], in_=wrkt_e[:])
            wrkt = cpool.tile([128, 256], F32R, tag="wrkt")
            nc.vector.tensor_copy(wrkt[:], wrkt_f[:])
            wrvta_f = cpool.tile([128, 65], F32, tag="wrvta_f")
            nc.sync.dma_start(out=wrvta_f[:], in_=wrvta_e[:])
            wrvtb_f = cpool.tile([4, 65], F32, tag="wrvtb_f")
            nc.sync.dma_start(out=wrvtb_f[:], in_=wrvtb_e[:])
            if with_mask_bias:
                mbp_f = cpool.tile([1, S + 128], F32, tag="mbp_f")
                nc.sync.dma_start(out=mbp_f[:], in_=mbp_e[:])
                mbp = cpool.tile([1, S + 128], F32R, tag="mbp")
                nc.vector.tensor_copy(mbp[:], mbp_f[:])
            mbc = cpool.tile([128, ST], F32, tag="mbc")
            nc.sync.dma_start(out=mbc[:], in_=mbc_e[:])
            iden_f = cpool.tile([128, 128], F32, tag="iden_f")
            nc.sync.dma_start(out=iden_f[:], in_=iden_e[:])
            insx = cpool.tile([128, 528], I16, tag="insx")
            nc.sync.dma_start(out=insx[:], in_=insx_e[:])
            valx2 = cpool.tile([128, 1024], I16, tag="valx2")
            nc.sync.dma_start(out=valx2[:], in_=valx_e[:])
            ones_f = cpool.tile([1, 128], F32, tag="ones_f")
            nc.sync.dma_start(out=ones_f[:], in_=ones_e[0:1, :])
            if with_mask_bias:
                ones = cpool.tile([1, 128], F32R, tag="ones")
                nc.vector.tensor_copy(ones[:], ones_f[:])
            bqc = cpool.tile([128, 4], F32, tag="bqc")
            nc.sync.dma_start(out=bqc[:], in_=bqc_e[:])
            bkc = cpool.tile([128, 4], F32, tag="bkc")
            nc.sync.dma_start(out=bkc[:], in_=bkc_e[:])

            iden = cpool.tile([128, 128], BF16, tag="iden")
            nc.vector.tensor_copy(iden[:], iden_f[:])
            ones_b = cpool.tile([1, 64], BF16, tag="ones_b")
            nc.vector.tensor_copy(ones_b[:], ones_f[0:1, 0:64])
            wrvt_a = cpool.tile([128, 65], BF16, tag="wrvt_a")
            nc.vector.tensor_copy(wrvt_a[:], wrvta_f[:])
            wrvt_b = cpool.tile([4, 65], BF16, tag="wrvt_b")
            nc.vector.tensor_copy(wrvt_b[:], wrvtb_f[:])

            # ---- inputs: x + weight chunks, k-interleaved so projections
            # start as soon as the first chunk lands ----
            xts = [xw.tile([128, S], F32R, tag=f"x{k}", name=f"x{k}") for k in range(KT)]
            wq = [xw.tile([128, 512], F32R, tag="w", bufs=8, name=f"wq{k}") for k in range(KT)]
            wk_ = [xw.tile([128, 512], F32R, tag="w", bufs=8, name=f"wk{k}") for k in range(KT)]
            wv = [xw.tile([128, 512], F32R, tag="w", bufs=8, name=f"wv{k}") for k in range(KT)]
            for k in range(KT):
                xs = xw.tile([128, S], F32, tag="xs", bufs=2, name=f"xs{k}")
                nc.sync.dma_start(out=xs[:], in_=xt_e[k * 128 : (k + 1) * 128, :])
                nc.vector.tensor_copy(xts[k][:], xs[:])
                ws = xw.tile([128, 512], F32, tag="ws", bufs=2, name=f"wqs{k}")
                nc.sync.dma_start(out=ws[:], in_=wqt_e[k * 128 : (k + 1) * 128, :])
                nc.vector.tensor_copy(wq[k][:], ws[:])
            for k in range(KT):
                ws = xw.tile([128, 512], F32, tag="ws", bufs=2, name=f"wks{k}")
                nc.sync.dma_start(out=ws[:], in_=wkt_e[k * 128 : (k + 1) * 128, :])
                nc.vector.tensor_copy(wk_[k][:], ws[:])
            for k in range(KT):
                ws = xw.tile([128, 512], F32, tag="ws", bufs=2, name=f"wvs{k}")
                nc.sync.dma_start(out=ws[:], in_=wvt_e[k * 128 : (k + 1) * 128, :])
                nc.vector.tensor_copy(wv[k][:], ws[:])

            # ---- persistent activations ----
            qt = [ppool.tile([128, S], F32R, tag=f"qt{t}", name=f"qt{t}") for t in range(4)]
            kt = [ppool.tile([128, S + 128], F32R, tag=f"kt{t}", name=f"kt{t}") for t in range(4)]
            vsb = [ppool.tile([128, 8 * 65], BF16, tag=f"v{j}", name=f"v{j}") for j in range(ST)]

            # zero k padding columns (64 each side)
            for t in range(4):
                nc.vector.memset(kt[t][:, 0:64].bitcast(F32), 0.0)
                nc.vector.memset(kt[t][:, S + 64 : S + 128].bitcast(F32), 0.0)

            # ---- projections (chunked k-outer: 4 PSUM groups at a time) ----
            w4 = {}
            a4d = {}

            def emit_ak_compute(hp, g):
                """a_k for head pair hp, i-chunk group g (4 chunks)."""
                tq = qt[hp]
                a4s = []
                for side in range(2):
                    hh = 2 * hp + side
                    a4 = smp.tile([128, 528], BF16, tag="a4", bufs=8,
                                  name=f"a4_{hh}_{g}")
                    a4s.append(a4)
                    a4d[(hh, g)] = a4
                for q in range(4):
                    it = g * 4 + q
                    aks = []
                    for side in range(2):
                        base = side * 64
                        akps = stp.tile([128, 256], F32, tag="st", name=f"ak{side}")
                        nc.tensor.matmul(
                            akps[:],
                            tq[base : base + 64, it * 128 : (it + 1) * 128],
                            wrkt[base : base + 64, :],
                            start=True, stop=True,
                        )
                        aks.append(akps)
                    for side in range(2):
                        nc.vector.tensor_copy(
                            a4s[side][:, q * 132 : (q + 1) * 132],
                            aks[side][:, 0:132],
                        )

            def emit_ak_scatter(hp, g):
                for side in range(2):
                    hh = 2 * hp + side
                    wt4 = wtp.tile([128, 4 * 384], BF16,
                                   tag="w4", bufs=8, name=f"w4_{hh}_{g}")
                    nc.gpsimd.local_scatter(
                        wt4[:], a4d[(hh, g)][:], insx[:],
                        channels=128, num_elems=4 * 384, num_idxs=528,
                    )
                    w4[(hh, g)] = wt4

            def emit_proj(wsrcs, kind):
                # kind: 'q' | 'k' | 'v'
                for chunk in range(2):
                    groups = list(range(chunk * 4, chunk * 4 + 4))
                    ps = {}
                    for i in groups:
                        ps[i] = stp.tile([128, 512], F32, tag="st",
                                         name=f"{kind}ps{i}")
                    for k in range(KT):
                        for i in groups:
                            if kind == 'v':
                                lh = xts[k][:, i * 128 : (i + 1) * 128]
                                rh = wsrcs[k][:]
                            else:
                                t, nch = i // 2, i % 2
                                lh = wsrcs[k][:, t * 128 : (t + 1) * 128]
                                rh = xts[k][:, nch * 512 : (nch + 1) * 512]
                            nc.tensor.matmul(
                                ps[i][:], lh, rh,
                                start=(k == 0), stop=(k == KT - 1),
                            )
                    for i in groups:
                        if kind == 'v':
                            dst = vsb[i][:].rearrange("p (h d) -> p h d", h=8, d=65)
                            nc.vector.tensor_copy(
                                dst[:, :, 0:64],
                                ps[i][:].rearrange("p (h d) -> p h d", h=8, d=64),
                            )
                            nc.vector.memset(dst[:, :, 64:65], 1.0)
                        else:
                            t, nch = i // 2, i % 2
                            if kind == 'q':
                                dst, off, biases = qt, 0, bqc
                            else:
                                dst, off, biases = kt, 64, bkc
                            nc.scalar.activation(
                                dst[t][:, off + nch * 512 : off + (nch + 1) * 512],
                                ps[i][:],
                                AF.Identity,
                                bias=biases[:, t : t + 1],
                            )
                    if kind == 'q':
                        # qt[2*chunk], qt[2*chunk+1] complete -> start a_k for
                        # those head pairs so the GPSIMD shears drain during
                        # the k/v projections. hp2/hp3 scatters are deferred
                        # into the attention loop (8-slot wt4 rotation).
                        for hp in (2 * chunk, 2 * chunk + 1):
                            for g in range(2):
                                emit_ak_compute(hp, g)
                                if hp < 2:
                                    emit_ak_scatter(hp, g)

            emit_proj(wq, 'q')
            emit_proj(wk_, 'k')

            # v-projection is emitted group-by-group interleaved into hp0's B
            # phase (it fills the PE while B throttles on Scalar exps).
            # Groups borrow the ctx PSUM slots, idle until hp0's C.
            def emit_vproj_group(i):
                vp = ctxp.tile([128, 512], F32, tag=f"ctx{i % 2}",
                               name=f"vps{i}")
                for k in range(KT):
                    nc.tensor.matmul(
                        vp[:], xts[k][:, i * 128 : (i + 1) * 128], wv[k][:],
                        start=(k == 0), stop=(k == KT - 1),
                    )
                dst = vsb[i][:].rearrange("p (h d) -> p h d", h=8, d=65)
                nc.vector.tensor_copy(
                    dst[:, :, 0:64],
                    vp[:].rearrange("p (h d) -> p h d", h=8, d=64),
                )
                nc.vector.memset(dst[:, :, 64:65], 1.0)

            def wslice(hh, it, c0, c1):
                return w4[(hh, it // 4)][:, (it % 4) * 384 + c0 : (it % 4) * 384 + c1]

            # ---- attention: 2-stage software pipeline ----
            # Stage B(hp): scoresT -> probsT (+ banded chains); stage C(hp):
            # PV + Wrv accumulation + normalization. C(hp) is interleaved into
            # B(hp+1)'s jt loop so PV matmuls fill the PE whenever B throttles
            # on the Scalar exps; hp0's B is filled with the v-projection
            # instead. Head A lives at partitions 0:64, head B at 64:128 of
            # the same qt/kt tiles, so every K=64 matmul is emitted as an A/B
            # pair targeting disjoint PE row-strips that execute concurrently.
            prev = None

            def pv_step(st8, jt):
                prs_p, pa_p, pb_p, ctxs_p, hp_p = st8
                for ich in range(2):
                    for side in range(2):
                        hh = 2 * hp_p + side
                        nc.tensor.matmul(
                            ctxs_p[side][:, ich * 512 : (ich + 1) * 512],
                            vsb[jt][:, hh * 65 : (hh + 1) * 65],
                            prs_p[side][jt][:, ich * 512 : (ich + 1) * 512],
                            start=(jt == 0),
                            stop=(jt == ST - 1 and ich == 0),
                        )
                if jt % 2 == 0:
                    return
                g2 = jt // 2
                pp2 = pb_p[g2]
                pb2s = []
                for side in range(2):
                    o = side * 528
                    ppt2 = stp.tile([4, 256], F32, tag="st",
                                    name=f"ppt{side}")
                    nc.tensor.matmul(ppt2[:, 0:128], pp2[:, o + 128 : o + 132],
                                     iden[:], start=True, stop=True)
                    nc.tensor.matmul(ppt2[:, 128:256], pp2[:, o + 392 : o + 396],
                                     iden[:], start=True, stop=True)
                    pb2 = smp.tile([4, 256], BF16, tag=f"pb{side}",
                                   bufs=2, name=f"pb{side}_{g2}")
                    nc.vector.tensor_copy(pb2[:], ppt2[:])
                    pb2s.append(pb2)
                for side in range(2):
                    nc.tensor.matmul(
                        ctxs_p[side][0:65, g2 * 256 : (g2 + 1) * 256],
                        wrvt_a[:], pa_p[(side, g2)][:], start=False, stop=False,
                    )
                for side in range(2):
                    nc.tensor.matmul(
                        ctxs_p[side][0:65, g2 * 256 : (g2 + 1) * 256],
                        wrvt_b[:], pb2s[side][:], start=False,
                        stop=(g2 == 3),
                    )

            def emit_E(st8):
                prs_p, pa_p, pb_p, ctxs_p, hp_p = st8
                for side in range(2):
                    hh = 2 * hp_p + side
                    cs = smp.tile([65, S], F32, tag=f"cs{side}", bufs=1)
                    nc.vector.tensor_copy(cs[:], ctxs_p[side][:])
                    s128 = smp.tile([128, 8], F32, tag=f"s128_{side}", bufs=1)
                    nc.scalar.dma_start(out=s128[:], in_=cs[64:65, :])
                    s128r = smp.tile([128, 8], F32, tag=f"s128r_{side}", bufs=1)
                    nc.vector.reciprocal(s128r[:], s128[:])
                    sb128 = smp.tile([128, 8], BF16, tag=f"sb128_{side}", bufs=1)
                    nc.vector.tensor_copy(sb128[:], s128r[:])
                    rbf = smp.tile([1, S], BF16, tag=f"rbf{side}", bufs=1)
                    nc.scalar.dma_start(out=rbf[0:1, :], in_=sb128[:])
                    rbc = smp.tile([64, S], BF16, tag="rbc", bufs=1,
                                   name=f"rbc{side}")
                    nc.gpsimd.partition_broadcast(rbc[:], rbf[0:1, :])
                    for ich in range(2):
                        nc.vector.tensor_mul(
                            cs[0:64, ich * 512 : (ich + 1) * 512],
                            cs[0:64, ich * 512 : (ich + 1) * 512],
                            rbc[:, ich * 512 : (ich + 1) * 512],
                        )
                    nc.scalar.dma_start(out=out_e[hh], in_=cs[0:64, :])

            for hp in range(4):
                tq = qt[hp]
                tk = kt[hp]

                prs = [[], []]
                pa = {}
                pb = {}

                def emit_band_chain(g2):
                    bss = []
                    for side in range(2):
                        bs2 = stp.tile([128, 512], F32, tag="st",
                                       name=f"bs{side}")
                        bss.append(bs2)
                    for h2 in range(2):
                        it = g2 * 2 + h2
                        for side in range(2):
                            base = side * 64
                            nc.tensor.matmul(
                                bss[side][:, h2 * 256 : (h2 + 1) * 256],
                                tq[base : base + 64, it * 128 : (it + 1) * 128],
                                tk[base : base + 64, it * 128 : it * 128 + 256],
                                start=True, stop=not with_mask_bias,
                            )
                        if with_mask_bias:
                            for side in range(2):
                                nc.tensor.matmul(
                                    bss[side][:, h2 * 256 : (h2 + 1) * 256],
                                    ones[0:1, :],
                                    mbp[0:1, it * 128 : it * 128 + 256],
                                    start=False, stop=True,
                                )
                    for h2 in range(2):
                        it = g2 * 2 + h2
                        for side in range(2):
                            hh = 2 * hp + side
                            nc.vector.tensor_add(
                                bss[side][:, h2 * 256 : (h2 + 1) * 256],
                                bss[side][:, h2 * 256 : (h2 + 1) * 256],
                                wslice(hh, it, 64, 320),
                            )
                    ppu2 = smp.tile([128, 1024], BF16, tag="ppu2")
                    for side in range(2):
                        nc.scalar.activation(
                            ppu2[:, side * 512 : (side + 1) * 512],
                            bss[side][:], AF.Exp,
                        )
                    if g2 == 0:
                        nc.vector.memset(ppu2[:, 0:64], 0.0)
                        nc.vector.memset(ppu2[:, 512:576], 0.0)
                    if g2 == 3:
                        nc.vector.memset(ppu2[:, 448:512], 0.0)
                        nc.vector.memset(ppu2[:, 960:1024], 0.0)
                    pp2 = smp.tile([128, 1056], BF16, tag="pp2", bufs=2,
                                   name=f"pp2_{g2}")
                    nc.gpsimd.local_scatter(
                        pp2[:], ppu2[:], valx2[:],
                        channels=128, num_elems=1056, num_idxs=1024,
                    )
                    for side in range(2):
                        o = side * 528
                        pa2 = smp.tile([128, 256], BF16, tag=f"pa{side}_{g2}",
                                       bufs=1, name=f"pa{side}_{g2}")
                        nc.sync.dma_start_transpose(pa2[:, 0:128], pp2[:, o : o + 128])
                        nc.sync.dma_start_transpose(pa2[:, 128:256], pp2[:, o + 264 : o + 392])
                        pa[(side, g2)] = pa2
                    pb[g2] = pp2

                ctxs = None
                for jt in range(ST):
                    for side in range(2):
                        pr = prp.tile([128, S], BF16, tag=f"pr{jt}{side}",
                                      name=f"pr{jt}{side}")
                        prs[side].append(pr)
                    exps = []
                    for ich in range(2):
                        pieces = [(jt, 128)]
                        if jt > 0:
                            pieces.append((jt - 1, 256))
                        if jt < ST - 1:
                            pieces.append((jt + 1, 0))
                        pieces = [p for p in pieces if p[0] // 4 == ich]
                        sts = []
                        for side in range(2):
                            base = side * 64
                            st = stp.tile([128, 512], F32, tag="st",
                                          name=f"st{side}")
                            sts.append(st)
                            nc.tensor.matmul(
                                st[:, 0:512],
                                tk[base : base + 64, 64 + jt * 128 : 64 + (jt + 1) * 128],
                                tq[base : base + 64, ich * 512 : (ich + 1) * 512],
                                start=True, stop=(not pieces),
                            )
                        for side in range(2):
                            hh = 2 * hp + side
                            for i, (src_it, c0) in enumerate(pieces):
                                lo = (src_it % 4) * 128
                                nc.tensor.matmul(
                                    sts[side][:, lo : lo + 128],
                                    wslice(hh, src_it, c0, c0 + 128),
                                    iden[:],
                                    start=False, stop=(i == len(pieces) - 1),
                                )
                        exps.append(sts)
                    # previous stage's PV for this jt (or a v-proj group in
                    # the prologue) fills the PE before this jt's exps gate
                    # the next groups.
                    if prev is not None:
                        if prev[3] is None:
                            ctx_p = []
                            for side in range(2):
                                ctx = ctxp.tile([65, S], F32, tag=f"ctx{side}",
                                                name=f"ctx{side}")
                                ctx_p.append(ctx)
                            prev = (prev[0], prev[1], prev[2], ctx_p, prev[4])
                        pv_step(prev, jt)
                    else:
                        emit_vproj_group(jt)
                    for ich in range(2):
                        for side in range(2):
                            nc.scalar.activation(
                                prs[side][jt][:, ich * 512 : (ich + 1) * 512],
                                exps[ich][side][:],
                                AF.Exp, bias=mbc[:, jt : jt + 1],
                            )
                    if jt % 2 == 0:
                        continue
                    g2 = jt // 2
                    emit_band_chain(g2)
                    # deferred hp+2 shear scatter: its wt4 slot is freed by
                    # this hp's band adds for the matching group, which just ran.
                    if hp < 2 and g2 in (1, 3):
                        emit_ak_scatter(hp + 2, g2 // 2)

                if prev is not None:
                    emit_E(prev)
                prev = (prs, pa, pb, None, hp)

            # epilogue: C + E for the last head pair
            ctx_p = []
            for side in range(2):
                ctx = ctxp.tile([65, S], F32, tag=f"ctx{side}", name=f"ctx{side}")
                ctx_p.append(ctx)
            prev = (prev[0], prev[1], prev[2], ctx_p, prev[4])
            for jt in range(ST):
                pv_step(prev, jt)
            emit_E(prev)

    nc.compile()
    return nc


_NC_CACHE = {}


def _get_nc(with_mask_bias=False):
    if with_mask_bias not in _NC_CACHE:
        _NC_CACHE[with_mask_bias] = _build_nc(with_mask_bias)
    return _NC_CACHE[with_mask_bias]


def _host_prep(inputs):
    hs = np.asarray(inputs["hidden_states"], np.float32)
    am = np.asarray(inputs["attention_mask"], np.float32)
    Wq = np.asarray(inputs["Wq"], np.float32)
    bq = np.asarray(inputs["bq"], np.float32)
    Wk = np.asarray(inputs["Wk"], np.float32)
    bk = np.asarray(inputs["bk"], np.float32)
    Wv = np.asarray(inputs["Wv"], np.float32)
    Wrk = np.asarray(inputs["Wrk"], np.float32)
    Wrv = np.asarray(inputs["Wrv"], np.float32)

    wrkt = np.zeros((128, 256), np.float32)
    wrkt[0:64, 0:NW] = Wrk.T
    wrkt[64:128, 0:NW] = Wrk.T
    wrvt = np.zeros((132, 65), np.float32)
    wrvt[0:NW, 0:64] = Wrv.T
    wrvt_a = np.ascontiguousarray(wrvt[0:128])
    wrvt_b = np.ascontiguousarray(wrvt[128:132])

    iden = np.eye(128, dtype=np.float32)
    p = np.arange(128)[:, None]
    j = np.arange(528)[None, :]
    q, w = j // 132, j % 132
    ins_idx = np.where(w <= 128, q * 384 + p + w + 64, -1).astype(np.int16)
    # side-batched un-shear: input [128, 2*512] (two sides' 2x256 windows),
    # output [128, 2*528]: pp2[p, side*528 + h*264 + (cc - p)]
    c = np.arange(1024)[None, :]
    side_i = c // 512
    cc_i = c % 512
    h, cc = cc_i // 256, cc_i % 256
    dd = cc - p
    val_idx2 = np.where((dd >= 0) & (dd <= 128),
                        side_i * 528 + h * 264 + dd, -1).astype(np.int16)
    ones_row = np.ones((128, 128), np.float32)

    in_maps = []
    for core in range(NCORES):
        b = core // 2
        h0 = (core % 2) * NHC
        fsl = slice(h0 * DH, h0 * DH + 512)
        mb = (1.0 - am[b]) * NEG
        mbp = np.zeros((1, S + 128), np.float32)
        mbp[0, 64 : 64 + S] = mb
        in_maps.append({
            "xt": np.ascontiguousarray(hs[b].T),
            "wqt": np.ascontiguousarray(Wq[fsl].T),
            "wkt": np.ascontiguousarray(Wk[fsl].T),
            "wvt": np.ascontiguousarray(Wv[fsl].T),
            "bq_cols": np.ascontiguousarray(bq[fsl].reshape(4, 128).T),
            "bk_cols": np.ascontiguousarray(bk[fsl].reshape(4, 128).T),
            "wrkt": wrkt,
            "wrvt_a": wrvt_a,
            "wrvt_b": wrvt_b,
            "mbias_pad": mbp,
            "mbias_cols": np.ascontiguousarray(mb.reshape(ST, 128).T),
            "identity": iden,
            "ins_idx": ins_idx,
            "val_idx2": val_idx2,
            "ones_row": ones_row,
        })
    return in_maps


def _assemble(results, inputs):
    bv = np.asarray(inputs["bv"], np.float32)
    full = np.empty((B, S, H * DH), np.float32)
    for core in range(NCORES):
        b = core // 2
        h0 = (core % 2) * NHC
        o = results[core]["out"]  # [NHC, DH, S]
        for hh in range(NHC):
            h = h0 + hh
            full[b, :, h * DH : (h + 1) * DH] = o[hh].T
    full += bv[None, None, :]
    return full


def kernel(**inputs):
    global LAST_EXEC_NS, LAST_RESULTS
    mask_all_ones = bool(np.all(np.asarray(inputs["attention_mask"]) == 1.0))
    nc = _get_nc(with_mask_bias=not mask_all_ones)
    in_maps = _host_prep(inputs)
    trace = bool(int(os.environ.get("KERNEL_TRACE", "0")))
    res = bass_utils.run_bass_kernel_spmd(
        nc, in_maps, core_ids=list(range(NCORES)), trace=trace
    )
    LAST_EXEC_NS = res.exec_time_ns
    LAST_RESULTS = res
    return _assemble(res.results, inputs)
